# revision 47
# baseline (speedup 1.0000x reference)
"""AttentionPooling (query position 0 only) — Trainium2 Bass/Tile kernel.

Math (per batch n, heads h=8, dh=32, D=256, T=4096):
    q0 = v[n,0,:] @ W_q + b_q                                  (256,)
    scores[t,h] = (1/16) * sum_{j in head h} q0[j] * k[t,j],   k = v @ W_k + b_k
Fold the k-projection into a per-batch "folded query":
    fq[din,h] = sum_{j in head h} W_k[din,j] * q0[j] / 16
    scores[t,h] = sum_din v[t,din] * fq[din,h]   (+ const(h) which CANCELS in
    softmax since it is uniform over t — so it is dropped entirely)
    out[h,:] = sum_t softmax_t(scores[:,h]) * v[t,:] -> keep cols [32h:32h+32]

Performance structure (per core: 4 batches, v shard 16.8 MB, HBM floor
~47us @358GB/s; measured single-ring DMA pace ~3.7us/MB):
  - All streaming matmuls in bf16 (1-pass PE + fast weight load; fp32 is
    2-pass). l2 rel err ~2.3e-3, well under the 2e-2 gate.
  - scores are computed TRANSPOSED [t,h] with v^T blocks as the stationary
    operand (weights) and the tiny fq (8 cols) streaming: LDW-dominated
    ~64cyc/block instead of streaming 128 cols. exp output then lands in
    natural t-major layout, so no exp-transpose is needed and the value
    matmul (again v as weights, e streaming 8 cols) consumes it directly.
  - v is DMA'd 1MB at a time in (p j) d layout: each partition line is 8KB
    contiguous HBM; chunk cadence is DMA-paced at ~3.9us.
  - PSUM discipline: every accumulation group is chunk-local and groups
    sharing a bank run strictly sequentially — a group START clears the
    has_written bits of its WHOLE bank, so interleaving two open groups in
    one bank silently turns accumulates into overwrites. Cross-chunk
    accumulation happens in SBUF (DVE adds).
  - the fp32->bf16 downcast for chunk i+1 is emitted before the compute of
    chunk i (1-stage software pipeline) so the PE does not wait on the
    scalar engine in steady state.
  - normalization (divide by Z) and per-head column extraction on host
    (tiny: 32x256 output).

Sharding: data-parallel over N across 8 cores (4 batches/core), no
collectives.
"""

import sys

if "/opt/trn_rl_repo" not in sys.path:
    sys.path.insert(0, "/opt/trn_rl_repo")

import numpy as np

N_FULL, T, DIN = 32, 4096, 256
H = 8
NCORES = 8
NB = N_FULL // NCORES  # batches per core
TC = 1024              # t-chunk per DMA + compute step
NJ = TC // 128         # rows per partition line (t = t0 + p*NJ + j)
NCH = T // TC          # chunks per batch
SCALE = 1.0 / 16.0     # 1/sqrt(D)

_CACHE = {}


def _build():
    from contextlib import ExitStack

    import concourse.mybir as mybir
    from concourse import bacc
    from concourse.masks import make_identity
    from concourse.tile import TileContext

    fp32 = mybir.dt.float32
    bf16 = mybir.dt.bfloat16
    AF = mybir.ActivationFunctionType

    nc = bacc.Bacc(None, target_bir_lowering=False)
    v_ext = nc.declare_dram_parameter("v", [NB, T, DIN], fp32, isOutput=False)
    w_ext = nc.declare_dram_parameter("W_qk", [DIN, 2 * DIN], fp32, isOutput=False)
    b_ext = nc.declare_dram_parameter("b_qk", [2 * DIN], fp32, isOutput=False)
    # unnormalized pooled values: acc[p, n, db*8+h] = sum_t e[t,h] v[t, db*128+p]
    acc_ext = nc.declare_dram_parameter("acc", [128, NB, 2 * H], fp32, isOutput=True)
    # softmax denominators, per (j, h) partial: z[0, n, j*8+h]
    z_ext = nc.declare_dram_parameter("z", [1, NB, NJ * H], fp32, isOutput=True)

    with TileContext(nc) as tc:
        with ExitStack() as ctx:
            const = ctx.enter_context(tc.tile_pool(name="const", bufs=1))

            ident = const.tile([128, 128], fp32)
            make_identity(nc, ident)
            ident_bf = const.tile([128, 128], bf16)
            nc.vector.tensor_copy(out=ident_bf, in_=ident)
            ones_f = const.tile([128, 1], fp32)
            nc.gpsimd.memset(ones_f, 1.0)
            ones_bf = const.tile([128, 1], bf16)
            nc.vector.tensor_copy(out=ones_bf, in_=ones_f)

            # W_q / W_k as [din_p, kc, dout] (kc = 128-chunk of din)
            wq_sb = const.tile([128, 2, 256], fp32)
            nc.sync.dma_start(
                out=wq_sb, in_=w_ext[:, 0:256].rearrange("(kc p) d -> p kc d", p=128)
            )
            wk_sb = const.tile([128, 2, 256], fp32)
            nc.sync.dma_start(
                out=wk_sb, in_=w_ext[:, 256:512].rearrange("(kc p) d -> p kc d", p=128)
            )
            bq_sb = const.tile([128, 2], fp32)
            nc.sync.dma_start(
                out=bq_sb, in_=b_ext[0:256].rearrange("(kc p) -> p kc", p=128)
            )
            # v0T[din_p, kc, n] = v[n, 0, :]
            v0_sb = const.tile([128, 2, NB], fp32)
            for kc in range(2):
                nc.sync.dma_start(
                    out=v0_sb[:, kc, :],
                    in_=v_ext[:, 0, kc * 128 : (kc + 1) * 128].rearrange("n p -> p n"),
                )

            # ---- phase 0: per-batch folded queries (all tiny, fp32) ----
            with tc.tile_pool(name="ps_prep", bufs=2, space="PSUM") as ps_prep:
                # WkT[j_p, jc, din] = W_k.T via PE transpose (matmul w/ identity)
                wkT_sb = const.tile([128, 2, 256], fp32)
                for jc in range(2):
                    pw = ps_prep.tile([128, 256], fp32, tag="pw")
                    for kc in range(2):
                        nc.tensor.matmul(
                            pw[:, kc * 128 : (kc + 1) * 128],
                            lhsT=wk_sb[:, kc, jc * 128 : (jc + 1) * 128],
                            rhs=ident,
                            start=True,
                            stop=True,
                        )
                    nc.vector.tensor_copy(out=wkT_sb[:, jc, :], in_=pw)

                # q0[dq_p, dqc, n] = W_q.T @ v0 + b_q   (batched over n)
                q0_sb = const.tile([128, 2, NB], fp32)
                for dqc in range(2):
                    pq = ps_prep.tile([128, NB], fp32, tag="pq")
                    for kc in range(2):
                        nc.tensor.matmul(
                            pq,
                            lhsT=wq_sb[:, kc, dqc * 128 : (dqc + 1) * 128],
                            rhs=v0_sb[:, kc, :],
                            start=(kc == 0),
                            stop=(kc == 1),
                        )
                    nc.scalar.activation(
                        out=q0_sb[:, dqc, :],
                        in_=pq,
                        func=AF.Identity,
                        bias=bq_sb[:, dqc : dqc + 1],
                        scale=1.0,
                    )

                # head mask[j_p, jc, h] = SCALE where j = 128*jc + j_p lies in
                # head h's 32-slice, else 0  (j - 32h in [0, 32))
                mask_sb = const.tile([128, 2, H], fp32)
                nc.gpsimd.memset(mask_sb, SCALE)
                nc.gpsimd.affine_select(
                    out=mask_sb,
                    in_=mask_sb,
                    compare_op=mybir.AluOpType.is_ge,
                    fill=0.0,
                    base=0,
                    pattern=[[128, 2], [-32, H]],
                    channel_multiplier=1,
                )
                nc.gpsimd.affine_select(
                    out=mask_sb,
                    in_=mask_sb,
                    compare_op=mybir.AluOpType.is_ge,
                    fill=0.0,
                    base=31,
                    pattern=[[-128, 2], [32, H]],
                    channel_multiplier=-1,
                )

                # q0m[j_p, jc, n*8+h] = mask * q0 (per-partition scalar)
                q0m_sb = const.tile([128, 2, NB * H], fp32)
                for n in range(NB):
                    for jc in range(2):
                        nc.vector.tensor_scalar_mul(
                            q0m_sb[:, jc, n * H : (n + 1) * H],
                            mask_sb[:, jc, :],
                            q0_sb[:, jc, n : n + 1],
                        )

                # fq[din_p, kc, n*8+h] = W_k @ q0m  (lhsT = WkT), cast to bf16
                fq_bf = const.tile([128, 2, NB * H], bf16)
                for kc in range(2):
                    pf = ps_prep.tile([128, NB * H], fp32, tag="pf")
                    for jc in range(2):
                        nc.tensor.matmul(
                            pf,
                            lhsT=wkT_sb[:, jc, kc * 128 : (kc + 1) * 128],
                            rhs=q0m_sb[:, jc, :],
                            start=(jc == 0),
                            stop=(jc == 1),
                        )
                    nc.vector.tensor_copy(out=fq_bf[:, kc, :], in_=pf)

            res_sb = const.tile([128, NB, 2 * H], fp32)
            zres_sb = const.tile([1, NB, NJ * H], fp32)

            # ---- phase 1: stream v ----
            vch = ctx.enter_context(tc.tile_pool(name="vch", bufs=3))
            vbp = ctx.enter_context(tc.tile_pool(name="vb", bufs=3))
            vtp = ctx.enter_context(tc.tile_pool(name="vt", bufs=2))
            epl = ctx.enter_context(tc.tile_pool(name="e", bufs=2))
            ps_vt = ctx.enter_context(tc.tile_pool(name="ps_vt", bufs=2, space="PSUM"))
            ps_st = ctx.enter_context(tc.tile_pool(name="ps_st", bufs=2, space="PSUM"))
            ps_vz = ctx.enter_context(tc.tile_pool(name="ps_vz", bufs=2, space="PSUM"))

            def load(n, ci):
                t0 = ci * TC
                vc = vch.tile([128, NJ, DIN], fp32, tag="vch", name="vc")
                # partition p <- rows t0+NJ*p .. t0+NJ*p+NJ-1: 8KB contiguous
                nc.sync.dma_start(
                    out=vc,
                    in_=v_ext[n, t0 : t0 + TC, :].rearrange("(p j) d -> p j d", p=128),
                )
                vb = vbp.tile([128, NJ, DIN], bf16, tag="vb", name="vb")
                nc.scalar.activation(out=vb, in_=vc, func=AF.Copy)
                return vb

            def process(n, ci, vb):
                # v^T for this chunk: vt[d_p, kc, j*128+p] (bf16 PSUM transposes)
                vt_sb = vtp.tile([128, 2, TC], bf16, tag="vt", name="vt_sb")
                for kc in range(2):
                    pv = ps_vt.tile([128, TC], bf16, tag="pv", name="pv")
                    for j in range(NJ):
                        nc.tensor.transpose(
                            pv[:, j * 128 : (j + 1) * 128],
                            in_=vb[:, j, kc * 128 : (kc + 1) * 128],
                            identity=ident_bf,
                        )
                    nc.vector.tensor_copy(out=vt_sb[:, kc, :], in_=pv)

                # scoresT[t_p, j*8+h]: v^T blocks stationary, fq streams (N=8)
                sT = ps_st.tile(
                    [128, NJ * H], fp32, tag="st", name="sT", padded_shape=[128, 512]
                )
                for j in range(NJ):
                    for kc in range(2):
                        nc.tensor.matmul(
                            sT[:, j * H : (j + 1) * H],
                            lhsT=vt_sb[:, kc, j * 128 : (j + 1) * 128],
                            rhs=fq_bf[:, kc, n * H : (n + 1) * H],
                            start=(kc == 0),
                            stop=(kc == 1),
                        )

                # e = exp(scores) in natural t-layout, bf16
                e_bf = epl.tile([128, NJ * H], bf16, tag="e", name="e_bf")
                nc.scalar.activation(out=e_bf, in_=sT, func=AF.Exp)

                # chunk-local value + denominator accumulators (one bank):
                # vz[:, db*8+h] = sum_t v[t, db*128+p] e[t, h]
                # vz[0, 16 + j*8+h] = sum_p e[p, j, h]
                vz = ps_vz.tile(
                    [128, 2 * H + NJ * H], fp32, tag="vz", name="vz",
                    padded_shape=[128, 512],
                )
                for db in range(2):
                    for j in range(NJ):
                        nc.tensor.matmul(
                            vz[:, db * H : (db + 1) * H],
                            lhsT=vb[:, j, db * 128 : (db + 1) * 128],
                            rhs=e_bf[:, j * H : (j + 1) * H],
                            start=(j == 0),
                            stop=(j == NJ - 1),
                        )
                nc.tensor.matmul(
                    vz[0:1, 2 * H : 2 * H + NJ * H],
                    lhsT=ones_bf,
                    rhs=e_bf,
                    start=True,
                    stop=True,
                )

                # cross-chunk accumulation in SBUF
                if ci == 0:
                    nc.vector.tensor_copy(out=res_sb[:, n, :], in_=vz[:, 0 : 2 * H])
                    nc.vector.tensor_copy(
                        out=zres_sb[:, n, :], in_=vz[0:1, 2 * H : 2 * H + NJ * H]
                    )
                else:
                    nc.vector.tensor_add(
                        out=res_sb[:, n, :],
                        in0=res_sb[:, n, :],
                        in1=vz[:, 0 : 2 * H],
                    )
                    nc.vector.tensor_add(
                        out=zres_sb[:, n, :],
                        in0=zres_sb[:, n, :],
                        in1=vz[0:1, 2 * H : 2 * H + NJ * H],
                    )
                if ci == NCH - 1:
                    # stream this batch's outputs out now (tiny) instead of
                    # waiting for the end — trims the drain tail
                    nc.sync.dma_start(
                        out=acc_ext[:, n : n + 1, :], in_=res_sb[:, n : n + 1, :]
                    )
                    nc.sync.dma_start(
                        out=z_ext[:, n : n + 1, :], in_=zres_sb[:, n : n + 1, :]
                    )

            # software pipeline: downcast for chunk i+1 is issued before the
            # compute of chunk i so the PE never waits on the scalar engine
            staged = None
            for n in range(NB):
                for ci in range(NCH):
                    vb = load(n, ci)
                    if staged is not None:
                        process(*staged)
                    staged = (n, ci, vb)
            process(*staged)

    nc.compile()
    return nc


def _get_nc():
    if "nc" not in _CACHE:
        _CACHE["nc"] = _build()
    return _CACHE["nc"]


def _run(inputs, trace=False):
    from concourse.bass_utils import run_bass_kernel_spmd

    v = np.ascontiguousarray(np.asarray(inputs["v"], dtype=np.float32))
    w = np.ascontiguousarray(np.asarray(inputs["W_qk"], dtype=np.float32))
    b = np.ascontiguousarray(np.asarray(inputs["b_qk"], dtype=np.float32))
    nc = _get_nc()
    in_maps = [
        {"v": v[c * NB : (c + 1) * NB], "W_qk": w, "b_qk": b} for c in range(NCORES)
    ]
    res = run_bass_kernel_spmd(nc, in_maps, list(range(NCORES)), trace=trace)

    d = np.arange(DIN)
    p, cb, h = d % 128, d // 128, d // 32
    out = np.empty((N_FULL, DIN), dtype=np.float32)
    for c in range(NCORES):
        acc = res.results[c]["acc"]          # (128, NB, 2*H)
        z = res.results[c]["z"][0]           # (NB, NJ*H)
        Z = z.reshape(NB, NJ, H).sum(axis=1)  # (NB, H)
        sel = acc[p, :, cb * H + h]          # (DIN, NB)
        out[c * NB : (c + 1) * NB] = (sel / Z[:, h].T).T
    return out, res


def kernel(**inputs) -> np.ndarray:
    return _run(inputs, trace=False)[0]
